# revision 8
# baseline (speedup 1.0000x reference)
"""Trainium2 Bass kernel for nn_BertTemporalOrdering.

Computes, per example b and span j over hidden [B,S,H]:
  mean_j = mean of hidden rows in span_j
  att_j  = softmax(q_j . tanh(W_j hidden + b_j))-weighted sum over span_j
  final  = concat([mean_0, att_0, ..., mean_4, att_4, full_emb]) per example
Returns (final [B, J*2H+H], full_emb) like the reference.

Strategy: data-parallel over B across 8 NeuronCores (2 examples/core),
load-balanced by span workload. One SPMD NEFF; per-core span-specialized
instruction streams live behind partition-id branches (RAGGED=True), so
each core only computes scores/projections inside its own spans (padded
to >=256 columns for full-rate float32r matmuls).
"""
import numpy as np
from contextlib import ExitStack

import ml_dtypes
import concourse.bass as bass
import concourse.tile as tile
from concourse import bacc, mybir
from concourse.bass_utils import run_bass_kernel_spmd

B, S, H, J = 16, 1024, 768, 5
NCORES, BPC = 8, 2
KT, ST, OT = H // 128, S // 128, H // 128
CHK = 512        # max matmul moving-dim chunk
MINW = 256       # min moving width for full-rate fp32r
NEG = -1e9
f32, f32r, bf16 = mybir.dt.float32, mybir.dt.float32r, mybir.dt.bfloat16
FA = mybir.ActivationFunctionType
ALU = mybir.AluOpType

RAGGED = True

_cache = {}


def _pad_range(st, en):
    """128-aligned range covering [st,en), width >= MINW, inside [0,S]."""
    sc_st = (st // 128) * 128
    sc_en = -(-en // 128) * 128
    while sc_en - sc_st < MINW:
        if sc_en < S:
            sc_en += 128
        else:
            sc_st -= 128
    return sc_st, sc_en


def _chunks(sc_st, sc_en):
    """Split into <=CHK pieces, each a multiple of 128."""
    nb = (sc_en - sc_st) // 128
    n = max(1, -(-nb // (CHK // 128)))
    base, rem = nb // n, nb % n
    out, pos = [], sc_st
    for i in range(n):
        w = (base + (1 if i < rem else 0)) * 128
        out.append((pos, pos + w))
        pos += w
    return out


def _emit_core(tc, nc_, sh, spans_core, tag):
    """Emit one core's compute with branch-local pools.

    sh: shared dict (read-only ht tiles, consts, dram handles).
    spans_core: [(st,en)] * (BPC*J).  tag: unique per emission."""
    with ExitStack() as ctx:
        w_pool = ctx.enter_context(tc.tile_pool(name=f"w{tag}", bufs=2))
        pjt_pool = ctx.enter_context(tc.tile_pool(name=f"pj{tag}", bufs=2))
        row_pool = ctx.enter_context(tc.tile_pool(name=f"rw{tag}", bufs=2))
        hid_pool = ctx.enter_context(tc.tile_pool(name=f"hd{tag}", bufs=12))
        loc_pool = ctx.enter_context(tc.tile_pool(name=f"lc{tag}", bufs=1))
        ps_mm = ctx.enter_context(tc.tile_pool(name=f"pm{tag}", bufs=3,
                                               space="PSUM"))
        ps_sc = ctx.enter_context(tc.tile_pool(name=f"ps{tag}", bufs=3,
                                               space="PSUM"))
        ps_tp = ctx.enter_context(tc.tile_pool(name=f"pt{tag}", bufs=2,
                                               space="PSUM"))
        ht, qc_t, bc_t, zc_t, eye1 = (sh["ht"], sh["qc"], sh["bc"],
                                      sh["zc"], sh["eye1"])

        # branch-local poolw (fm data + alpha columns), packed on host
        poolw = []
        for e in range(BPC):
            pw = loc_pool.tile([128, ST, 2 * J], f32r, name=f"pw{tag}{e}")
            nc_.gpsimd.dma_start(
                pw[:], sh["fm"][e].rearrange("p (t c) -> p t c", c=2 * J))
            poolw.append(pw)

        # prefetch natural-layout hidden for pooling (gpsimd DMA queue)
        hid_tiles = {}
        for e in range(BPC):
            tiles = sorted({t for (stj, enj) in spans_core[e * J:(e + 1) * J]
                            if enj > stj
                            for t in range(stj // 128, -(-enj // 128))})
            for t in (tiles or [0]):
                hd = hid_pool.tile([128, H], f32r, tag="hid",
                                   name=f"hid{tag}{e}{t}")
                nc_.gpsimd.dma_start(hd[:],
                                     sh["hid"][e, t * 128:(t + 1) * 128, :])
                hid_tiles[(e, t)] = hd

        # stage 2 of the (j,e) pipeline: scores + softmax + alpha transposes.
        # Deferred one iteration so the PE never stalls on the DVE/ACT chain.
        def _stage2(j, e, st, en, sc_st, sc_en, chunks, pjt):
            T = sc_en - sc_st
            srow = row_pool.tile([1, 1024], f32, tag="srow",
                                 name=f"sr{tag}{j}{e}")
            erow = row_pool.tile([1, 1024], f32, tag="erow",
                                 name=f"er{tag}{j}{e}")
            mrt = row_pool.tile([1, 1024], bf16, tag="mrt",
                                name=f"mr{tag}{j}{e}")
            nc_.gpsimd.dma_start(mrt[:, :T], sh["mr"][e, j, sc_st:sc_en])
            for (c0, c1) in chunks:
                sc_ps = ps_sc.tile([1, CHK], f32, tag="sc",
                                   name=f"sc{tag}{j}{e}")
                for oi in range(OT):
                    nc_.tensor.matmul(
                        sc_ps[:, : c1 - c0],
                        qc_t[:, j * OT + oi:j * OT + oi + 1],
                        pjt[:, oi, c0 - sc_st:c1 - sc_st],
                        start=(oi == 0), stop=(oi == OT - 1))
                nc_.vector.tensor_tensor(
                    srow[:, c0 - sc_st:c1 - sc_st], sc_ps[:, : c1 - c0],
                    mrt[:, c0 - sc_st:c1 - sc_st], op=ALU.add)
            negmax = row_pool.tile([1, 1], f32, tag="nm",
                                   name=f"nm{tag}{j}{e}")
            nc_.vector.tensor_reduce(negmax[:], srow[:, :T],
                                     axis=mybir.AxisListType.X,
                                     op=ALU.max, negate=True)
            sumc = row_pool.tile([1, 1], f32, tag="sm",
                                 name=f"sm{tag}{j}{e}")
            nc_.scalar.activation(erow[:, :T], srow[:, :T], FA.Exp,
                                  bias=negmax[:], accum_out=sumc[:])
            rec = row_pool.tile([1, 1], f32, tag="rc",
                                name=f"rc{tag}{j}{e}")
            nc_.vector.reciprocal(rec[:], sumc[:])
            nc_.vector.tensor_scalar(erow[:, :T], erow[:, :T], rec[:],
                                     zc_t[:, e * J + j:e * J + j + 1],
                                     op0=ALU.mult, op1=ALU.mult)
            # only tiles overlapping the true span need alphas copied
            t0, t1 = st // 128, -(-en // 128)
            for t in range(t0, t1):
                a0 = t * 128
                tp = ps_tp.tile([128, 1], f32, tag="tp",
                                name=f"tp{tag}{j}{e}{t}")
                nc_.tensor.transpose(tp[:],
                                     erow[:, a0 - sc_st:a0 + 128 - sc_st],
                                     eye1[:])
                nc_.vector.tensor_copy(
                    poolw[e][:, t, 2 * j + 1:2 * j + 2], tp[:])

        # j outer: each weight slab loaded once (branch-local DMA), used for
        # both examples, then released.
        pending = None
        for j in range(J):
            todo = [e for e in range(BPC)
                    if spans_core[e * J + j][1] > spans_core[e * J + j][0]]
            if not todo:
                continue
            wj = w_pool.tile([128, KT, H], f32r, tag="wj", name=f"wj{tag}{j}")
            wsrc = sh["wl"][:, j * H:(j + 1) * H].rearrange(
                "(k p) o -> k p o", p=128)
            for k in range(KT):
                nc_.sync.dma_start(wj[:, k], wsrc[k])
            for e in todo:
                st, en = spans_core[e * J + j]
                sc_st, sc_en = _pad_range(st, en)
                chunks = _chunks(sc_st, sc_en)

                pjt = pjt_pool.tile([128, OT, 1024], f32r, tag="pjt",
                                    name=f"pjt{tag}{j}{e}")
                for oi in range(OT):
                    for (c0, c1) in chunks:
                        pt = ps_mm.tile([128, CHK], f32, tag="mm",
                                        name=f"pt{tag}{j}{e}{oi}")
                        for k in range(KT):
                            nc_.tensor.matmul(
                                pt[:, : c1 - c0],
                                wj[:, k, oi * 128:(oi + 1) * 128],
                                ht[e][:, k, c0:c1],
                                start=(k == 0), stop=(k == KT - 1))
                        nc_.scalar.activation(
                            pjt[:, oi, c0 - sc_st:c1 - sc_st],
                            pt[:, : c1 - c0], FA.Tanh,
                            bias=bc_t[:, j * OT + oi:j * OT + oi + 1])
                if pending is not None:
                    pending()
                args = (j, e, st, en, sc_st, sc_en, chunks, pjt)
                pending = lambda a=args: _stage2(*a)
        if pending is not None:
            pending()

        # pooling over s-tiles covered by any span of this example
        for e in range(BPC):
            tiles = sorted({t for (stj, enj) in spans_core[e * J:(e + 1) * J]
                            if enj > stj
                            for t in range(stj // 128, -(-enj // 128))})
            if not tiles:
                tiles = [0]
            pp0 = ps_mm.tile([10, 384], f32, tag="mm", name=f"pp0{tag}{e}")
            pp1 = ps_mm.tile([10, 384], f32, tag="mm", name=f"pp1{tag}{e}")
            for i, t in enumerate(tiles):
                hd = hid_tiles[(e, t)]
                nc_.tensor.matmul(pp0[:], poolw[e][:, t, :], hd[:, 0:384],
                                  start=(i == 0), stop=(i == len(tiles) - 1))
                nc_.tensor.matmul(pp1[:], poolw[e][:, t, :], hd[:, 384:768],
                                  start=(i == 0), stop=(i == len(tiles) - 1))
            osb = loc_pool.tile([10, 768], f32, name=f"osb{tag}{e}")
            nc_.vector.tensor_copy(osb[:, 0:384], pp0[:])
            nc_.vector.tensor_copy(osb[:, 384:768], pp1[:])
            nc_.sync.dma_start(
                sh["of"][e, 0:J * 2 * H].rearrange("(r o) -> r o", o=H),
                osb[:])


def _build(spans_by_core):
    nc = bacc.Bacc("TRN2", target_bir_lowering=False, debug=False,
                   num_devices=NCORES)

    hT_d = nc.dram_tensor("hT", [BPC, H, S], f32r, kind="ExternalInput").ap()
    hid_d = nc.dram_tensor("hid", [BPC, S, H], f32r, kind="ExternalInput").ap()
    wl_d = nc.dram_tensor("wl", [H, J * H], f32r, kind="ExternalInput").ap()
    qc_d = nc.dram_tensor("qc", [128, J * OT], f32r, kind="ExternalInput").ap()
    bc_d = nc.dram_tensor("bc", [128, J * OT], f32, kind="ExternalInput").ap()
    fm_d = nc.dram_tensor("fm", [BPC, 128, ST * 2 * J], f32r,
                          kind="ExternalInput").ap()
    zc_d = nc.dram_tensor("zc", [1, BPC * J], f32, kind="ExternalInput").ap()
    mr_d = nc.dram_tensor("mr", [BPC, J, S], bf16, kind="ExternalInput").ap()
    fe_d = nc.dram_tensor("fe", [BPC, H], f32, kind="ExternalInput").ap()

    of_d = nc.dram_tensor("out_final", [BPC, J * 2 * H + H], f32,
                          kind="ExternalOutput").ap()
    oe_d = nc.dram_tensor("out_emb", [BPC, H], f32, kind="ExternalOutput").ap()

    with tile.TileContext(nc) as tc:
        with ExitStack() as ctx:
            const_pool = ctx.enter_context(tc.tile_pool(name="consts", bufs=1))
            ht_pool = ctx.enter_context(tc.tile_pool(name="ht", bufs=1))

            qc_t = const_pool.tile([128, J * OT], f32r, name="qc_t")
            nc.sync.dma_start(qc_t[:], qc_d[:])
            bc_t = const_pool.tile([128, J * OT], f32, name="bc_t")
            nc.sync.dma_start(bc_t[:], bc_d[:])
            zc_t = const_pool.tile([1, BPC * J], f32, name="zc_t")
            nc.sync.dma_start(zc_t[:], zc_d[:])
            eye1 = const_pool.tile([1, 1], f32, name="eye1")
            nc.vector.memset(eye1[:], 1.0)

            ht = []
            for e in range(BPC):
                ht_e = ht_pool.tile([128, KT, S], f32r, name=f"ht_{e}")
                src = hT_d[e].rearrange("(k p) s -> k p s", p=128)
                for k in range(KT):
                    nc.gpsimd.dma_start(ht_e[:, k], src[k])
                ht.append(ht_e)

            nc.sync.dma_start(oe_d[:], fe_d[:])
            for e in range(BPC):
                nc.sync.dma_start(of_d[e, J * 2 * H:], fe_d[e, :])

            sh = dict(ht=ht, qc=qc_t, bc=bc_t, zc=zc_t, eye1=eye1,
                      wl=wl_d, fm=fm_d, mr=mr_d, hid=hid_d, of=of_d)

            if RAGGED:
                pid = nc.partition_id()

                def tree(lo, hi):
                    if hi - lo == 1:
                        _emit_core(tc, nc, sh, spans_by_core[lo], f"c{lo}")
                        return
                    mid = (lo + hi) // 2
                    with tc.If(pid < mid) as cmp:
                        tree(lo, mid)
                    with cmp.Else():
                        tree(mid, hi)

                tree(0, NCORES)
            else:
                _emit_core(tc, nc, sh, [(0, S)] * (BPC * J), "u")

    nc.compile()
    return nc


def _balance(sp):
    """Pair examples to cores to minimize the max per-core padded workload.
    Returns perm[16]: perm[c*BPC+e] = original example index."""
    w = []
    for b in range(B):
        tot = 0
        for j in range(J):
            st, en = int(sp[b, j, 0]), int(sp[b, j, 1])
            if en > st:
                s0, s1 = _pad_range(st, en)
                tot += s1 - s0
        w.append((tot, b))
    w.sort(reverse=True)
    order = [b for _, b in w]
    perm = []
    lo, hi = 0, B - 1
    while lo < hi:
        perm.extend([order[lo], order[hi]])
        lo += 1
        hi -= 1
    if lo == hi:
        perm.append(order[lo])
    return perm


def _host_pack(hidden, full_emb, attn_W, attn_b, attn_q, spans):
    hidden = np.ascontiguousarray(hidden, dtype=np.float32)
    full_emb = np.ascontiguousarray(full_emb, dtype=np.float32)
    attn_W = np.ascontiguousarray(attn_W, dtype=np.float32)
    attn_b = np.ascontiguousarray(attn_b, dtype=np.float32)
    attn_q = np.ascontiguousarray(attn_q, dtype=np.float32)
    sp = np.asarray(spans).astype(np.int64)

    perm = _balance(sp) if RAGGED else list(range(B))

    wl = np.ascontiguousarray(attn_W.transpose(2, 0, 1).reshape(H, J * H))
    qc = np.ascontiguousarray(attn_q.reshape(J, OT, 128).transpose(2, 0, 1)
                              .reshape(128, J * OT))
    bc = np.ascontiguousarray(attn_b.reshape(J, OT, 128).transpose(2, 0, 1)
                              .reshape(128, J * OT))

    in_maps, spans_by_core = [], []
    for c in range(NCORES):
        bs = [perm[c * BPC + e] for e in range(BPC)]
        hT = np.ascontiguousarray(hidden[bs].transpose(0, 2, 1))
        hid = np.ascontiguousarray(hidden[bs])
        fe = np.ascontiguousarray(full_emb[bs])
        fm = np.zeros((BPC, S, 2 * J), np.float32)
        zc = np.zeros((1, BPC * J), np.float32)
        mr = np.full((BPC, J, S), NEG, np.float32)
        spl = []
        for e in range(BPC):
            b = bs[e]
            for j in range(J):
                st, en = int(sp[b, j, 0]), int(sp[b, j, 1])
                spl.append((st, en))
                if en > st:
                    fm[e, st:en, 2 * j] = 1.0 / (en - st)
                    zc[0, e * J + j] = 1.0
                    mr[e, j, st:en] = 0.0
        spans_by_core.append(spl)
        fmp = np.ascontiguousarray(
            fm.reshape(BPC, ST, 128, 2 * J).transpose(0, 2, 1, 3)
            .reshape(BPC, 128, ST * 2 * J))
        in_maps.append({"hT": hT, "hid": hid, "wl": wl, "qc": qc, "bc": bc,
                        "fm": fmp, "zc": zc,
                        "mr": mr.astype(ml_dtypes.bfloat16), "fe": fe})
    return in_maps, spans_by_core, perm


def kernel(hidden, full_emb, attn_W, attn_b, attn_q, spans, _trace=False):
    in_maps, spans_by_core, perm = _host_pack(hidden, full_emb, attn_W,
                                              attn_b, attn_q, spans)
    key = (RAGGED, np.asarray(spans).astype(np.int64).tobytes())
    if key not in _cache:
        _cache[key] = _build(spans_by_core)
    nc = _cache[key]

    res = run_bass_kernel_spmd(nc, in_maps, core_ids=list(range(NCORES)),
                               trace=_trace)
    global LAST_RESULT
    LAST_RESULT = res

    final = np.empty((B, J * 2 * H + H), np.float32)
    emb = np.empty((B, H), np.float32)
    for c in range(NCORES):
        for e in range(BPC):
            b = perm[c * BPC + e]
            final[b] = res.results[c]["out_final"][e]
            emb[b] = res.results[c]["out_emb"][e]
    return final, emb


LAST_RESULT = None


# revision 9
# speedup vs baseline: 1.0630x; 1.0630x over previous
"""Trainium2 Bass kernel for nn_BertTemporalOrdering.

Computes, per example b and span j over hidden [B,S,H]:
  mean_j = mean of hidden rows in span_j
  att_j  = softmax(q_j . tanh(W_j hidden + b_j))-weighted sum over span_j
  final  = concat([mean_0, att_0, ..., mean_4, att_4, full_emb]) per example
Returns (final [B, J*2H+H], full_emb) like the reference.

Strategy: data-parallel over B across 8 NeuronCores (2 examples/core),
load-balanced by span workload. One SPMD NEFF; per-core span-specialized
instruction streams live behind partition-id branches (RAGGED=True), so
each core only computes scores/projections inside its own spans (padded
to >=256 columns for full-rate float32r matmuls).
"""
import numpy as np
from contextlib import ExitStack

import ml_dtypes
import concourse.bass as bass
import concourse.tile as tile
from concourse import bacc, mybir
from concourse.bass_utils import run_bass_kernel_spmd

B, S, H, J = 16, 1024, 768, 5
NCORES, BPC = 8, 2
KT, ST, OT = H // 128, S // 128, H // 128
CHK = 512        # max matmul moving-dim chunk
MINW = 256       # min moving width for full-rate fp32r
NEG = -1e9
f32, f32r, bf16 = mybir.dt.float32, mybir.dt.float32r, mybir.dt.bfloat16
FA = mybir.ActivationFunctionType
ALU = mybir.AluOpType

RAGGED = True

_cache = {}


def _pad_range(st, en):
    """128-aligned range covering [st,en), width >= MINW, inside [0,S]."""
    sc_st = (st // 128) * 128
    sc_en = -(-en // 128) * 128
    while sc_en - sc_st < MINW:
        if sc_en < S:
            sc_en += 128
        else:
            sc_st -= 128
    return sc_st, sc_en


def _chunks(sc_st, sc_en):
    """Split into <=CHK pieces, each a multiple of 128."""
    nb = (sc_en - sc_st) // 128
    n = max(1, -(-nb // (CHK // 128)))
    base, rem = nb // n, nb % n
    out, pos = [], sc_st
    for i in range(n):
        w = (base + (1 if i < rem else 0)) * 128
        out.append((pos, pos + w))
        pos += w
    return out


def _emit_core(tc, nc_, sh, spans_core, tag):
    """Emit one core's compute with branch-local pools.

    sh: shared dict (read-only ht tiles, consts, dram handles).
    spans_core: [(st,en)] * (BPC*J).  tag: unique per emission."""
    with ExitStack() as ctx:
        w_pool = ctx.enter_context(tc.tile_pool(name=f"w{tag}", bufs=2))
        pjt_pool = ctx.enter_context(tc.tile_pool(name=f"pj{tag}", bufs=2))
        row_pool = ctx.enter_context(tc.tile_pool(name=f"rw{tag}", bufs=2))
        hid_pool = ctx.enter_context(tc.tile_pool(name=f"hd{tag}", bufs=12))
        loc_pool = ctx.enter_context(tc.tile_pool(name=f"lc{tag}", bufs=1))
        ps_mm = ctx.enter_context(tc.tile_pool(name=f"pm{tag}", bufs=3,
                                               space="PSUM"))
        ps_sc = ctx.enter_context(tc.tile_pool(name=f"ps{tag}", bufs=3,
                                               space="PSUM"))
        ps_tp = ctx.enter_context(tc.tile_pool(name=f"pt{tag}", bufs=2,
                                               space="PSUM"))
        ht, qc_t, bc_t, zc_t, eye1 = (sh["ht"], sh["qc"], sh["bc"],
                                      sh["zc"], sh["eye1"])

        # branch-local poolw (fm data + alpha columns), packed on host
        poolw = []
        for e in range(BPC):
            pw = loc_pool.tile([128, ST, 2 * J], f32r, name=f"pw{tag}{e}")
            nc_.gpsimd.dma_start(
                pw[:], sh["fm"][e].rearrange("p (t c) -> p t c", c=2 * J))
            poolw.append(pw)


        # stage 2 of the (j,e) pipeline: scores + softmax + alpha transposes.
        # Deferred one iteration so the PE never stalls on the DVE/ACT chain.
        def _stage2(j, e, st, en, sc_st, sc_en, chunks, pjt):
            T = sc_en - sc_st
            srow = row_pool.tile([1, 1024], f32, tag="srow",
                                 name=f"sr{tag}{j}{e}")
            erow = row_pool.tile([1, 1024], f32, tag="erow",
                                 name=f"er{tag}{j}{e}")
            mrt = row_pool.tile([1, 1024], bf16, tag="mrt",
                                name=f"mr{tag}{j}{e}")
            nc_.gpsimd.dma_start(mrt[:, :T], sh["mr"][e, j, sc_st:sc_en])
            for (c0, c1) in chunks:
                sc_ps = ps_sc.tile([1, CHK], f32, tag="sc",
                                   name=f"sc{tag}{j}{e}")
                for oi in range(OT):
                    nc_.tensor.matmul(
                        sc_ps[:, : c1 - c0],
                        qc_t[:, j * OT + oi:j * OT + oi + 1],
                        pjt[:, oi, c0 - sc_st:c1 - sc_st],
                        start=(oi == 0), stop=(oi == OT - 1))
                nc_.vector.tensor_tensor(
                    srow[:, c0 - sc_st:c1 - sc_st], sc_ps[:, : c1 - c0],
                    mrt[:, c0 - sc_st:c1 - sc_st], op=ALU.add)
            negmax = row_pool.tile([1, 1], f32, tag="nm",
                                   name=f"nm{tag}{j}{e}")
            nc_.vector.tensor_reduce(negmax[:], srow[:, :T],
                                     axis=mybir.AxisListType.X,
                                     op=ALU.max, negate=True)
            sumc = row_pool.tile([1, 1], f32, tag="sm",
                                 name=f"sm{tag}{j}{e}")
            nc_.scalar.activation(erow[:, :T], srow[:, :T], FA.Exp,
                                  bias=negmax[:], accum_out=sumc[:])
            rec = row_pool.tile([1, 1], f32, tag="rc",
                                name=f"rc{tag}{j}{e}")
            nc_.vector.reciprocal(rec[:], sumc[:])
            nc_.vector.tensor_scalar(erow[:, :T], erow[:, :T], rec[:],
                                     zc_t[:, e * J + j:e * J + j + 1],
                                     op0=ALU.mult, op1=ALU.mult)
            # only tiles overlapping the true span need alphas copied
            t0, t1 = st // 128, -(-en // 128)
            for t in range(t0, t1):
                a0 = t * 128
                tp = ps_tp.tile([128, 1], f32, tag="tp",
                                name=f"tp{tag}{j}{e}{t}")
                nc_.tensor.transpose(tp[:],
                                     erow[:, a0 - sc_st:a0 + 128 - sc_st],
                                     eye1[:])
                nc_.vector.tensor_copy(
                    poolw[e][:, t, 2 * j + 1:2 * j + 2], tp[:])

        # j outer: each weight slab loaded once (branch-local DMA), used for
        # both examples, then released.
        pending = None
        for j in range(J):
            todo = [e for e in range(BPC)
                    if spans_core[e * J + j][1] > spans_core[e * J + j][0]]
            if not todo:
                continue
            wj = w_pool.tile([128, KT, H], f32r, tag="wj", name=f"wj{tag}{j}")
            wsrc = sh["wl"][:, j * H:(j + 1) * H].rearrange(
                "(k p) o -> k p o", p=128)
            for k in range(KT):
                nc_.sync.dma_start(wj[:, k], wsrc[k])
            for e in todo:
                st, en = spans_core[e * J + j]
                sc_st, sc_en = _pad_range(st, en)
                chunks = _chunks(sc_st, sc_en)

                pjt = pjt_pool.tile([128, OT, 1024], f32r, tag="pjt",
                                    name=f"pjt{tag}{j}{e}")
                for oi in range(OT):
                    for (c0, c1) in chunks:
                        pt = ps_mm.tile([128, CHK], f32, tag="mm",
                                        name=f"pt{tag}{j}{e}{oi}")
                        for k in range(KT):
                            nc_.tensor.matmul(
                                pt[:, : c1 - c0],
                                wj[:, k, oi * 128:(oi + 1) * 128],
                                ht[e][:, k, c0:c1],
                                start=(k == 0), stop=(k == KT - 1))
                        nc_.scalar.activation(
                            pjt[:, oi, c0 - sc_st:c1 - sc_st],
                            pt[:, : c1 - c0], FA.Tanh,
                            bias=bc_t[:, j * OT + oi:j * OT + oi + 1])
                if pending is not None:
                    pending()
                args = (j, e, st, en, sc_st, sc_en, chunks, pjt)
                pending = lambda a=args: _stage2(*a)
        # natural-layout hidden for pooling: prefetch on the sync queue
        # behind the weight slabs (arrives well before pooling needs it)
        hid_tiles = {}
        for e in range(BPC):
            tiles = sorted({t for (stj, enj) in spans_core[e * J:(e + 1) * J]
                            if enj > stj
                            for t in range(stj // 128, -(-enj // 128))})
            for t in (tiles or [0]):
                hd = hid_pool.tile([128, H], f32r, tag="hid",
                                   name=f"hid{tag}{e}{t}")
                nc_.sync.dma_start(hd[:],
                                   sh["hid"][e, t * 128:(t + 1) * 128, :])
                hid_tiles[(e, t)] = hd

        if pending is not None:
            pending()

        # pooling over s-tiles covered by any span of this example
        for e in range(BPC):
            tiles = sorted({t for (stj, enj) in spans_core[e * J:(e + 1) * J]
                            if enj > stj
                            for t in range(stj // 128, -(-enj // 128))})
            if not tiles:
                tiles = [0]
            pp0 = ps_mm.tile([10, 384], f32, tag="mm", name=f"pp0{tag}{e}")
            pp1 = ps_mm.tile([10, 384], f32, tag="mm", name=f"pp1{tag}{e}")
            for i, t in enumerate(tiles):
                hd = hid_tiles[(e, t)]
                nc_.tensor.matmul(pp0[:], poolw[e][:, t, :], hd[:, 0:384],
                                  start=(i == 0), stop=(i == len(tiles) - 1))
                nc_.tensor.matmul(pp1[:], poolw[e][:, t, :], hd[:, 384:768],
                                  start=(i == 0), stop=(i == len(tiles) - 1))
            osb = loc_pool.tile([10, 768], f32, name=f"osb{tag}{e}")
            nc_.vector.tensor_copy(osb[:, 0:384], pp0[:])
            nc_.vector.tensor_copy(osb[:, 384:768], pp1[:])
            nc_.sync.dma_start(
                sh["of"][e, 0:J * 2 * H].rearrange("(r o) -> r o", o=H),
                osb[:])


def _build(spans_by_core):
    nc = bacc.Bacc("TRN2", target_bir_lowering=False, debug=False,
                   num_devices=NCORES)

    hT_d = nc.dram_tensor("hT", [BPC, H, S], f32r, kind="ExternalInput").ap()
    hid_d = nc.dram_tensor("hid", [BPC, S, H], f32r, kind="ExternalInput").ap()
    wl_d = nc.dram_tensor("wl", [H, J * H], f32r, kind="ExternalInput").ap()
    qc_d = nc.dram_tensor("qc", [128, J * OT], f32r, kind="ExternalInput").ap()
    bc_d = nc.dram_tensor("bc", [128, J * OT], f32, kind="ExternalInput").ap()
    fm_d = nc.dram_tensor("fm", [BPC, 128, ST * 2 * J], f32r,
                          kind="ExternalInput").ap()
    zc_d = nc.dram_tensor("zc", [1, BPC * J], f32, kind="ExternalInput").ap()
    mr_d = nc.dram_tensor("mr", [BPC, J, S], bf16, kind="ExternalInput").ap()
    fe_d = nc.dram_tensor("fe", [BPC, H], f32, kind="ExternalInput").ap()

    of_d = nc.dram_tensor("out_final", [BPC, J * 2 * H + H], f32,
                          kind="ExternalOutput").ap()
    oe_d = nc.dram_tensor("out_emb", [BPC, H], f32, kind="ExternalOutput").ap()

    with tile.TileContext(nc) as tc:
        with ExitStack() as ctx:
            const_pool = ctx.enter_context(tc.tile_pool(name="consts", bufs=1))
            ht_pool = ctx.enter_context(tc.tile_pool(name="ht", bufs=1))

            qc_t = const_pool.tile([128, J * OT], f32r, name="qc_t")
            nc.sync.dma_start(qc_t[:], qc_d[:])
            bc_t = const_pool.tile([128, J * OT], f32, name="bc_t")
            nc.sync.dma_start(bc_t[:], bc_d[:])
            zc_t = const_pool.tile([1, BPC * J], f32, name="zc_t")
            nc.sync.dma_start(zc_t[:], zc_d[:])
            eye1 = const_pool.tile([1, 1], f32, name="eye1")
            nc.vector.memset(eye1[:], 1.0)

            ht = []
            for e in range(BPC):
                ht_e = ht_pool.tile([128, KT, S], f32r, name=f"ht_{e}")
                src = hT_d[e].rearrange("(k p) s -> k p s", p=128)
                for k in range(KT):
                    nc.gpsimd.dma_start(ht_e[:, k], src[k])
                ht.append(ht_e)

            nc.sync.dma_start(oe_d[:], fe_d[:])
            for e in range(BPC):
                nc.sync.dma_start(of_d[e, J * 2 * H:], fe_d[e, :])

            sh = dict(ht=ht, qc=qc_t, bc=bc_t, zc=zc_t, eye1=eye1,
                      wl=wl_d, fm=fm_d, mr=mr_d, hid=hid_d, of=of_d)

            if RAGGED:
                pid = nc.partition_id()

                def tree(lo, hi):
                    if hi - lo == 1:
                        _emit_core(tc, nc, sh, spans_by_core[lo], f"c{lo}")
                        return
                    mid = (lo + hi) // 2
                    with tc.If(pid < mid) as cmp:
                        tree(lo, mid)
                    with cmp.Else():
                        tree(mid, hi)

                tree(0, NCORES)
            else:
                _emit_core(tc, nc, sh, [(0, S)] * (BPC * J), "u")

    nc.compile()
    return nc


def _balance(sp):
    """Pair examples to cores to minimize the max per-core padded workload.
    Returns perm[16]: perm[c*BPC+e] = original example index."""
    w = []
    for b in range(B):
        tot = 0
        for j in range(J):
            st, en = int(sp[b, j, 0]), int(sp[b, j, 1])
            if en > st:
                s0, s1 = _pad_range(st, en)
                tot += s1 - s0
        w.append((tot, b))
    w.sort(reverse=True)
    order = [b for _, b in w]
    perm = []
    lo, hi = 0, B - 1
    while lo < hi:
        perm.extend([order[lo], order[hi]])
        lo += 1
        hi -= 1
    if lo == hi:
        perm.append(order[lo])
    return perm


def _host_pack(hidden, full_emb, attn_W, attn_b, attn_q, spans):
    hidden = np.ascontiguousarray(hidden, dtype=np.float32)
    full_emb = np.ascontiguousarray(full_emb, dtype=np.float32)
    attn_W = np.ascontiguousarray(attn_W, dtype=np.float32)
    attn_b = np.ascontiguousarray(attn_b, dtype=np.float32)
    attn_q = np.ascontiguousarray(attn_q, dtype=np.float32)
    sp = np.asarray(spans).astype(np.int64)

    perm = _balance(sp) if RAGGED else list(range(B))

    wl = np.ascontiguousarray(attn_W.transpose(2, 0, 1).reshape(H, J * H))
    qc = np.ascontiguousarray(attn_q.reshape(J, OT, 128).transpose(2, 0, 1)
                              .reshape(128, J * OT))
    bc = np.ascontiguousarray(attn_b.reshape(J, OT, 128).transpose(2, 0, 1)
                              .reshape(128, J * OT))

    in_maps, spans_by_core = [], []
    for c in range(NCORES):
        bs = [perm[c * BPC + e] for e in range(BPC)]
        hT = np.ascontiguousarray(hidden[bs].transpose(0, 2, 1))
        hid = np.ascontiguousarray(hidden[bs])
        fe = np.ascontiguousarray(full_emb[bs])
        fm = np.zeros((BPC, S, 2 * J), np.float32)
        zc = np.zeros((1, BPC * J), np.float32)
        mr = np.full((BPC, J, S), NEG, np.float32)
        spl = []
        for e in range(BPC):
            b = bs[e]
            for j in range(J):
                st, en = int(sp[b, j, 0]), int(sp[b, j, 1])
                spl.append((st, en))
                if en > st:
                    fm[e, st:en, 2 * j] = 1.0 / (en - st)
                    zc[0, e * J + j] = 1.0
                    mr[e, j, st:en] = 0.0
        spans_by_core.append(spl)
        fmp = np.ascontiguousarray(
            fm.reshape(BPC, ST, 128, 2 * J).transpose(0, 2, 1, 3)
            .reshape(BPC, 128, ST * 2 * J))
        in_maps.append({"hT": hT, "hid": hid, "wl": wl, "qc": qc, "bc": bc,
                        "fm": fmp, "zc": zc,
                        "mr": mr.astype(ml_dtypes.bfloat16), "fe": fe})
    return in_maps, spans_by_core, perm


def kernel(hidden, full_emb, attn_W, attn_b, attn_q, spans, _trace=False):
    in_maps, spans_by_core, perm = _host_pack(hidden, full_emb, attn_W,
                                              attn_b, attn_q, spans)
    key = (RAGGED, np.asarray(spans).astype(np.int64).tobytes())
    if key not in _cache:
        _cache[key] = _build(spans_by_core)
    nc = _cache[key]

    res = run_bass_kernel_spmd(nc, in_maps, core_ids=list(range(NCORES)),
                               trace=_trace)
    global LAST_RESULT
    LAST_RESULT = res

    final = np.empty((B, J * 2 * H + H), np.float32)
    emb = np.empty((B, H), np.float32)
    for c in range(NCORES):
        for e in range(BPC):
            b = perm[c * BPC + e]
            final[b] = res.results[c]["out_final"][e]
            emb[b] = res.results[c]["out_emb"][e]
    return final, emb


LAST_RESULT = None
